# revision 57
# baseline (speedup 1.0000x reference)
"""Box-SDF (CAPUDF box boundary distance) Trainium2 Bass kernel.

For each 3-D point x (S = 0.4):
    q  = |x| - S
    d  = sqrt(sum_i relu(q_i)^2)    if any q_i >= 0   (outside)
    d  = -max_i q_i                 otherwise         (inside)

Branch-free on-chip formulation:
    a_i = |x_i|                      (ScalarE Abs)
    mx  = max(a_0, a_1, a_2)         (VectorE max tree over planes)
    u   = min(mx, S) - S             (<= 0; = -inside-distance)
    b_i = max(a_i, S) - S            (relu(q_i), in place over a)
    s   = b_0^2 + b_1^2 + b_2^2      (Square on ScalarE/VectorE + adds)
    d   = sqrt(s) - u

The host pre-transposes each tile to planar component layout so every
on-chip access is contiguous (strided DVE access is ~8x slow, strided
ACT ~1.8x). Work is spread across ACT / DVE / POOL and emitted as a
3-stage software pipeline (per-engine instruction streams execute in
order, so tile t+1's front work must be emitted before tile t's tail).
Sharding: data-parallel over the points axis across 8 NeuronCores.
"""

import sys

import numpy as np

sys.path.insert(0, "/opt/trn_rl_repo")

import concourse.bacc as bacc  # noqa: E402
import concourse.mybir as mybir  # noqa: E402
from concourse import bass_utils  # noqa: E402
from concourse.tile import TileContext  # noqa: E402

N = 8388608
NCORES = 8
NPC = N // NCORES  # 1,048,576 points per core
P = 128
K = 1024  # points per partition row per tile
F3 = 3 * K  # floats per partition row per tile
NT = NPC // (P * K)  # 8 tiles per core

SIZE = 0.4
F32 = mybir.dt.float32
AF = mybir.ActivationFunctionType
OP = mybir.AluOpType


def build_kernel():
    nc = bacc.Bacc(
        "TRN2",
        target_bir_lowering=False,
        debug=False,
        num_devices=NCORES,
    )
    x = nc.dram_tensor("x", [NT, P, F3], F32, kind="ExternalInput").ap()
    eye = nc.dram_tensor("eye", [P, P], F32, kind="ExternalInput").ap()
    d = nc.dram_tensor("d", [NT, P, K], F32, kind="ExternalOutput").ap()

    with TileContext(nc) as tc:
        with (
            tc.tile_pool(name="const", bufs=1) as cpool,
            tc.tile_pool(name="xtp", bufs=4) as xtp,
            tc.tile_pool(name="big", bufs=3) as big,
            tc.tile_pool(name="small", bufs=3) as small,
            tc.tile_pool(name="psum", bufs=2, space="PSUM") as pspool,
        ):
            eye_t = cpool.tile([P, P], F32)
            state = {}

            def stage_a(t):
                xt = xtp.tile([P, F3], F32, tag="xt")
                a = big.tile([P, F3], F32, tag="a")
                if t == 0:
                    # Chunk the first tile per-plane so Abs starts sooner.
                    for c in range(3):
                        cs = slice(c * K, (c + 1) * K)
                        nc.sync.dma_start(out=xt[:, cs], in_=x[t][:, cs])
                        nc.scalar.activation(
                            out=a[:, cs], in_=xt[:, cs], func=AF.Abs
                        )
                else:
                    nc.sync.dma_start(out=xt[:], in_=x[t])
                    nc.scalar.activation(out=a[:], in_=xt[:], func=AF.Abs)

                # mx = max_i a_i (contiguous plane max tree)
                m1 = small.tile([P, K], F32, tag="m1_s01")
                nc.vector.tensor_tensor(
                    out=m1[:], in0=a[:, 0:K], in1=a[:, K : 2 * K], op=OP.max
                )
                mx = small.tile([P, K], F32, tag="mx_rt")
                nc.vector.tensor_tensor(
                    out=mx[:], in0=m1[:], in1=a[:, 2 * K : 3 * K], op=OP.max
                )

                # u = min(mx, S) - S  (<= 0; equals -(inside distance))
                u = small.tile([P, K], F32, tag="u")
                nc.vector.tensor_scalar(
                    out=u[:],
                    in0=mx[:],
                    scalar1=SIZE,
                    scalar2=-SIZE,
                    op0=OP.min,
                    op1=OP.add,
                )

                # b = relu(a - S) = max(a, S) - S  (contiguous, DVE 2x mode)
                b = big.tile([P, F3], F32, tag="b")
                nc.vector.tensor_scalar(
                    out=b[:],
                    in0=a[:],
                    scalar1=SIZE,
                    scalar2=-SIZE,
                    op0=OP.max,
                    op1=OP.add,
                )
                state[t] = (b, u)

            def stage_b(t):
                b, u = state.pop(t)
                # sq = b^2: planes 0,1 on ScalarE; plane 2 on VectorE
                sq = big.tile([P, F3], F32, tag="sq")
                if t % 2 == 0:
                    nc.scalar.activation(
                        out=sq[:, 0 : 2 * K], in_=b[:, 0 : 2 * K], func=AF.Square
                    )
                    nc.gpsimd.tensor_tensor(
                        out=sq[:, 2 * K : 3 * K],
                        in0=b[:, 2 * K : 3 * K],
                        in1=b[:, 2 * K : 3 * K],
                        op=OP.mult,
                    )
                else:
                    nc.scalar.activation(out=sq[:], in_=b[:], func=AF.Square)

                # s = sq0 + sq1 + sq2 via identity matmuls accumulating in
                # PSUM (TensorE is otherwise idle; PSUM accumulate = free add)
                s_ps = pspool.tile([P, K], F32, tag="s_ps")
                for j in range(0, K, 512):
                    for c in range(3):
                        nc.tensor.matmul(
                            s_ps[:, j : j + 512],
                            eye_t[:],
                            sq[:, c * K + j : c * K + j + 512],
                            start=(c == 0),
                            stop=(c == 2),
                        )
                # rt = sqrt(s)  (ScalarE reads PSUM directly)
                rt = small.tile([P, K], F32, tag="mx_rt")
                nc.scalar.activation(out=rt[:], in_=s_ps[:], func=AF.Sqrt)

                # d = rt - u: rt when outside (u=0), S-mx when inside (rt=0).
                # Last tile on DVE (faster op) to shorten the kernel tail.
                dt = small.tile([P, K], F32, tag="s_dt")
                eng = nc.vector if t == NT - 1 else nc.gpsimd
                eng.tensor_tensor(out=dt[:], in0=rt[:], in1=u[:], op=OP.subtract)

                nc.sync.dma_start(out=d[t], in_=dt[:])

            # 2-stage software pipeline emission: A(t+1) before B(t) so each
            # engine's in-order stream never stalls tile t+1's front work
            # behind tile t's tail work.
            stage_a(0)
            nc.sync.dma_start(out=eye_t[:], in_=eye[:])
            for t in range(1, NT):
                stage_a(t)
                stage_b(t - 1)
            stage_b(NT - 1)

    nc.compile()
    return nc


_cached_nc = None


def _get_nc():
    global _cached_nc
    if _cached_nc is None:
        _cached_nc = build_kernel()
    return _cached_nc


_AXON_SO = "/opt/axon/libaxon_pjrt.so"


def _ensure_ntff_hook():
    """Install an antenv.axon_hooks shim backed by libaxon_pjrt's NRT
    profiling C ABI, so run_bass_kernel_spmd(trace=True) works under axon."""
    try:
        from antenv.axon_hooks import get_axon_ntff_profile_hook  # noqa: F401

        return
    except ImportError:
        pass
    import contextlib
    import ctypes
    import types

    import antenv

    holder = {}
    mod = types.ModuleType("antenv.axon_hooks")
    mod.set_axon_ntff_profile_hook = lambda h: holder.__setitem__("h", h)
    mod.get_axon_ntff_profile_hook = lambda: holder.get("h")
    sys.modules["antenv.axon_hooks"] = mod
    antenv.axon_hooks = mod

    try:
        lib = ctypes.CDLL(_AXON_SO)
    except OSError:
        return
    if not hasattr(lib, "axon_start_nrt_profile"):
        return
    lib.axon_start_nrt_profile.argtypes = [
        ctypes.POINTER(ctypes.c_int64),
        ctypes.c_size_t,
    ]
    lib.axon_start_nrt_profile.restype = ctypes.c_int64
    lib.axon_stop_nrt_profile.argtypes = [ctypes.c_char_p]
    lib.axon_stop_nrt_profile.restype = ctypes.c_int64

    @contextlib.contextmanager
    def _hook(output_dir, device_ids):
        import jax

        jax.devices()
        if device_ids:
            ids = (ctypes.c_int64 * len(device_ids))(*device_ids)
            rc = lib.axon_start_nrt_profile(ids, len(device_ids))
        else:
            rc = lib.axon_start_nrt_profile(None, 0)
        if rc != 0:
            raise RuntimeError(f"axon_start_nrt_profile rc={rc}")
        try:
            yield
        finally:
            n = lib.axon_stop_nrt_profile(str(output_dir).encode())
            print(f"ntff profile: {n} file(s) written to {output_dir}")

    holder["h"] = _hook


def run(inputs_array, trace=False, **kwargs):
    """inputs_array: [N, 3] float32. Returns (out [N] float32, BassKernelResults)."""
    pts = np.ascontiguousarray(inputs_array, dtype=np.float32)
    assert pts.shape == (N, 3), pts.shape
    # Host-side de-interleave to planar: [NC, NT, P, K, 3] -> [NC, NT, P, 3, K]
    shards = np.ascontiguousarray(
        pts.reshape(NCORES, NT, P, K, 3).transpose(0, 1, 2, 4, 3)
    ).reshape(NCORES, NT, P, F3)
    if trace:
        _ensure_ntff_hook()
    nc = _get_nc()
    eye_np = np.eye(P, dtype=np.float32)
    in_maps = [{"x": shards[i], "eye": eye_np} for i in range(NCORES)]
    res = bass_utils.run_bass_kernel_spmd(
        nc, in_maps, core_ids=list(range(NCORES)), trace=trace, **kwargs
    )
    out = np.concatenate(
        [res.results[i]["d"].reshape(-1) for i in range(NCORES)]
    )
    return out, res


def kernel(**inputs):
    out, _ = run(inputs["inputs"])
    return out


if __name__ == "__main__":
    rng = np.random.default_rng(0)
    pts = rng.standard_normal((N, 3)).astype(np.float32)
    out, _ = run(pts)
    q = np.abs(pts) - SIZE
    inside = np.all(q < 0, axis=1)
    d_out = np.sqrt(np.sum(np.square(np.maximum(q, 0.0)), axis=1))
    d_in = -np.max(q, axis=1)
    exp = np.where(inside, d_in, d_out)
    err = np.abs(out - exp) / np.maximum(np.abs(exp), 1e-6)
    print("max rel err:", err.max(), "mean:", err.mean())


# revision 58
# speedup vs baseline: 1.0507x; 1.0507x over previous
"""Box-SDF (CAPUDF box boundary distance) Trainium2 Bass kernel.

For each 3-D point x (S = 0.4):
    q  = |x| - S
    d  = sqrt(sum_i relu(q_i)^2)    if any q_i >= 0   (outside)
    d  = -max_i q_i                 otherwise         (inside)

Branch-free on-chip formulation:
    a_i = |x_i|                      (ScalarE Abs)
    mx  = max(a_0, a_1, a_2)         (VectorE max tree over planes)
    u   = min(mx, S) - S             (<= 0; = -inside-distance)
    b_i = max(a_i, S) - S            (relu(q_i), in place over a)
    s   = b_0^2 + b_1^2 + b_2^2      (Square on ScalarE/VectorE + adds)
    d   = sqrt(s) - u

The host pre-transposes each tile to planar component layout so every
on-chip access is contiguous (strided DVE access is ~8x slow, strided
ACT ~1.8x). Work is spread across ACT / DVE / POOL and emitted as a
3-stage software pipeline (per-engine instruction streams execute in
order, so tile t+1's front work must be emitted before tile t's tail).
Sharding: data-parallel over the points axis across 8 NeuronCores.
"""

import sys

import numpy as np

sys.path.insert(0, "/opt/trn_rl_repo")

import concourse.bacc as bacc  # noqa: E402
import concourse.mybir as mybir  # noqa: E402
from concourse import bass_utils  # noqa: E402
from concourse.tile import TileContext  # noqa: E402

N = 8388608
NCORES = 8
NPC = N // NCORES  # 1,048,576 points per core
P = 128
K = 1024  # points per partition row per tile
F3 = 3 * K  # floats per partition row per tile
NT = NPC // (P * K)  # 8 tiles per core

SIZE = 0.4
F32 = mybir.dt.float32
AF = mybir.ActivationFunctionType
OP = mybir.AluOpType


def build_kernel():
    nc = bacc.Bacc(
        "TRN2",
        target_bir_lowering=False,
        debug=False,
        num_devices=NCORES,
    )
    x = nc.dram_tensor("x", [NT, P, F3], F32, kind="ExternalInput").ap()
    eye = nc.dram_tensor("eye", [P, P], F32, kind="ExternalInput").ap()
    d = nc.dram_tensor("d", [NT, P, K], F32, kind="ExternalOutput").ap()

    with TileContext(nc) as tc:
        with (
            tc.tile_pool(name="const", bufs=1) as cpool,
            tc.tile_pool(name="xtp", bufs=4) as xtp,
            tc.tile_pool(name="big", bufs=3) as big,
            tc.tile_pool(name="small", bufs=3) as small,
            tc.tile_pool(name="psum", bufs=2, space="PSUM") as pspool,
        ):
            eye_t = cpool.tile([P, P], F32)
            state = {}

            def stage_a(t):
                xt = xtp.tile([P, F3], F32, tag="xt")
                a = big.tile([P, F3], F32, tag="a")
                if t == 0:
                    # Chunk the first tile per-plane so Abs starts sooner.
                    for c in range(3):
                        cs = slice(c * K, (c + 1) * K)
                        nc.sync.dma_start(out=xt[:, cs], in_=x[t][:, cs])
                        nc.scalar.activation(
                            out=a[:, cs], in_=xt[:, cs], func=AF.Abs
                        )
                else:
                    nc.sync.dma_start(out=xt[:], in_=x[t])
                    nc.scalar.activation(out=a[:], in_=xt[:], func=AF.Abs)

                # mx = max_i a_i (contiguous plane max tree)
                m1 = small.tile([P, K], F32, tag="m1_s01")
                nc.vector.tensor_tensor(
                    out=m1[:], in0=a[:, 0:K], in1=a[:, K : 2 * K], op=OP.max
                )
                mx = small.tile([P, K], F32, tag="mx_rt")
                nc.vector.tensor_tensor(
                    out=mx[:], in0=m1[:], in1=a[:, 2 * K : 3 * K], op=OP.max
                )

                # u = min(mx, S) - S  (<= 0; equals -(inside distance))
                u = small.tile([P, K], F32, tag="u")
                nc.vector.tensor_scalar(
                    out=u[:],
                    in0=mx[:],
                    scalar1=SIZE,
                    scalar2=-SIZE,
                    op0=OP.min,
                    op1=OP.add,
                )

                # b = relu(a - S) = max(a, S) - S  (contiguous, DVE 2x mode)
                b = big.tile([P, F3], F32, tag="b")
                nc.vector.tensor_scalar(
                    out=b[:],
                    in0=a[:],
                    scalar1=SIZE,
                    scalar2=-SIZE,
                    op0=OP.max,
                    op1=OP.add,
                )
                state[t] = (b, u)

            def stage_b(t):
                b, u = state.pop(t)
                # sq = b^2: planes 0,1 on ScalarE; plane 2 on VectorE
                sq = big.tile([P, F3], F32, tag="sq")
                if t % 2 == 0:
                    nc.scalar.activation(
                        out=sq[:, 0 : 2 * K], in_=b[:, 0 : 2 * K], func=AF.Square
                    )
                    nc.vector.tensor_tensor(
                        out=sq[:, 2 * K : 3 * K],
                        in0=b[:, 2 * K : 3 * K],
                        in1=b[:, 2 * K : 3 * K],
                        op=OP.mult,
                    )
                else:
                    nc.scalar.activation(out=sq[:], in_=b[:], func=AF.Square)

                # s = sq0 + sq1 + sq2 via identity matmuls accumulating in
                # PSUM (TensorE is otherwise idle; PSUM accumulate = free add)
                s_ps = pspool.tile([P, K], F32, tag="s_ps")
                for j in range(0, K, 512):
                    for c in range(3):
                        nc.tensor.matmul(
                            s_ps[:, j : j + 512],
                            eye_t[:],
                            sq[:, c * K + j : c * K + j + 512],
                            start=(c == 0),
                            stop=(c == 2),
                        )
                # rt = sqrt(s)  (ScalarE reads PSUM directly)
                rt = small.tile([P, K], F32, tag="mx_rt")
                nc.scalar.activation(out=rt[:], in_=s_ps[:], func=AF.Sqrt)

                # d = rt - u: rt when outside (u=0), S-mx when inside (rt=0).
                # Last tile on DVE (faster op) to shorten the kernel tail.
                dt = small.tile([P, K], F32, tag="s_dt")
                eng = nc.vector if t == NT - 1 else nc.gpsimd
                eng.tensor_tensor(out=dt[:], in0=rt[:], in1=u[:], op=OP.subtract)

                nc.sync.dma_start(out=d[t], in_=dt[:])

            # 2-stage software pipeline emission: A(t+1) before B(t) so each
            # engine's in-order stream never stalls tile t+1's front work
            # behind tile t's tail work.
            stage_a(0)
            nc.sync.dma_start(out=eye_t[:], in_=eye[:])
            for t in range(1, NT):
                stage_a(t)
                stage_b(t - 1)
            stage_b(NT - 1)

    nc.compile()
    return nc


_cached_nc = None


def _get_nc():
    global _cached_nc
    if _cached_nc is None:
        _cached_nc = build_kernel()
    return _cached_nc


_AXON_SO = "/opt/axon/libaxon_pjrt.so"


def _ensure_ntff_hook():
    """Install an antenv.axon_hooks shim backed by libaxon_pjrt's NRT
    profiling C ABI, so run_bass_kernel_spmd(trace=True) works under axon."""
    try:
        from antenv.axon_hooks import get_axon_ntff_profile_hook  # noqa: F401

        return
    except ImportError:
        pass
    import contextlib
    import ctypes
    import types

    import antenv

    holder = {}
    mod = types.ModuleType("antenv.axon_hooks")
    mod.set_axon_ntff_profile_hook = lambda h: holder.__setitem__("h", h)
    mod.get_axon_ntff_profile_hook = lambda: holder.get("h")
    sys.modules["antenv.axon_hooks"] = mod
    antenv.axon_hooks = mod

    try:
        lib = ctypes.CDLL(_AXON_SO)
    except OSError:
        return
    if not hasattr(lib, "axon_start_nrt_profile"):
        return
    lib.axon_start_nrt_profile.argtypes = [
        ctypes.POINTER(ctypes.c_int64),
        ctypes.c_size_t,
    ]
    lib.axon_start_nrt_profile.restype = ctypes.c_int64
    lib.axon_stop_nrt_profile.argtypes = [ctypes.c_char_p]
    lib.axon_stop_nrt_profile.restype = ctypes.c_int64

    @contextlib.contextmanager
    def _hook(output_dir, device_ids):
        import jax

        jax.devices()
        if device_ids:
            ids = (ctypes.c_int64 * len(device_ids))(*device_ids)
            rc = lib.axon_start_nrt_profile(ids, len(device_ids))
        else:
            rc = lib.axon_start_nrt_profile(None, 0)
        if rc != 0:
            raise RuntimeError(f"axon_start_nrt_profile rc={rc}")
        try:
            yield
        finally:
            n = lib.axon_stop_nrt_profile(str(output_dir).encode())
            print(f"ntff profile: {n} file(s) written to {output_dir}")

    holder["h"] = _hook


def run(inputs_array, trace=False, **kwargs):
    """inputs_array: [N, 3] float32. Returns (out [N] float32, BassKernelResults)."""
    pts = np.ascontiguousarray(inputs_array, dtype=np.float32)
    assert pts.shape == (N, 3), pts.shape
    # Host-side de-interleave to planar: [NC, NT, P, K, 3] -> [NC, NT, P, 3, K]
    shards = np.ascontiguousarray(
        pts.reshape(NCORES, NT, P, K, 3).transpose(0, 1, 2, 4, 3)
    ).reshape(NCORES, NT, P, F3)
    if trace:
        _ensure_ntff_hook()
    nc = _get_nc()
    eye_np = np.eye(P, dtype=np.float32)
    in_maps = [{"x": shards[i], "eye": eye_np} for i in range(NCORES)]
    res = bass_utils.run_bass_kernel_spmd(
        nc, in_maps, core_ids=list(range(NCORES)), trace=trace, **kwargs
    )
    out = np.concatenate(
        [res.results[i]["d"].reshape(-1) for i in range(NCORES)]
    )
    return out, res


def kernel(**inputs):
    out, _ = run(inputs["inputs"])
    return out


if __name__ == "__main__":
    rng = np.random.default_rng(0)
    pts = rng.standard_normal((N, 3)).astype(np.float32)
    out, _ = run(pts)
    q = np.abs(pts) - SIZE
    inside = np.all(q < 0, axis=1)
    d_out = np.sqrt(np.sum(np.square(np.maximum(q, 0.0)), axis=1))
    d_in = -np.max(q, axis=1)
    exp = np.where(inside, d_in, d_out)
    err = np.abs(out - exp) / np.maximum(np.abs(exp), 1e-6)
    print("max rel err:", err.max(), "mean:", err.mean())
